# revision 24
# baseline (speedup 1.0000x reference)
"""Multi-head attention Trainium2 Bass kernel.

Problem: B=4, S=2048, H=16, DH=64, D=1024, fp32.
  q/k/v = hidden @ W{q,k,v}.T + b; scores = q k^T / 8; probs = softmax;
  ctx = probs v; out = ctx @ Wo.T + bo.

Sharding: tensor-parallel over heads. Core c owns heads (2c, 2c+1) for all
4 batches: column-slices of Wq/Wk/Wv (torch [out,in] layout -> row slices),
row-slice of Wo. Each core computes a partial output projection over its
128 context features; host sums the 8 partials and adds bo.

Per-core data layout (all matmuls in float32r, TF32-grade, full PE rate):
  h^T   [e=1024 (8 chunks of 128p), tok]  via PE transpose of hidden tiles
  Q^T/K^T [f=128 (2 heads x 64), tok]     = Wslice^T-lhsT x h^T-rhs
  Vaug  [tok (16 chunks of 128p), chunk, head, 65] (ones col + 64 V cols)
  S^T   [kt chunk 128p, q 1024]  = K^T-lhsT x Q^T-rhs     (PSUM)
  P     = exp(S^T)  (no max subtraction needed: scores ~ N(0,1))  (SBUF f32r)
  den/ctx^T = Vaug-lhsT x P-rhs accumulated over kt chunks (PSUM [65, q])
  ctxn^T = ctx^T * (1/den)  (den broadcast over partitions via GpSimd)
  out_partial [tok, fo] = ctxn^T-lhsT x Wo^T-rhs
"""
import numpy as np

import concourse.bass as bass
import concourse.tile as tile
from concourse import bacc, mybir
from concourse import bass_utils
from concourse.masks import make_identity

F32 = mybir.dt.float32
F32R = mybir.dt.float32r
EXP = mybir.ActivationFunctionType.Exp

B = 4
S = 2048
D = 1024
NCORES = 8
P = 128          # partitions
EC = D // P      # 8 e-chunks
TT = S // 512    # 4 token tiles per batch
KC = S // P      # 16 kt chunks
QB = S // 1024   # 2 q blocks per head
F = 128          # per-core feature slice (2 heads x 64)


def build_nc():
    nc = bacc.Bacc("TRN2", target_bir_lowering=False, debug=False,
                   enable_asserts=True, num_devices=NCORES)

    hid = nc.dram_tensor("hid", [B, S, D], F32, kind="ExternalInput").ap()
    wq = nc.dram_tensor("wq", [F, D], F32, kind="ExternalInput").ap()
    wk = nc.dram_tensor("wk", [F, D], F32, kind="ExternalInput").ap()
    wv = nc.dram_tensor("wv", [F, D], F32, kind="ExternalInput").ap()
    wo = nc.dram_tensor("wo", [D, F], F32, kind="ExternalInput").ap()
    bq = nc.dram_tensor("bq", [F], F32, kind="ExternalInput").ap()
    bk = nc.dram_tensor("bk", [F], F32, kind="ExternalInput").ap()
    bv = nc.dram_tensor("bv", [F], F32, kind="ExternalInput").ap()
    out = nc.dram_tensor("out", [B, S, D], F32, kind="ExternalOutput").ap()

    with tile.TileContext(nc) as tc:
        with (
            tc.tile_pool(name="const", bufs=1) as cpool,
            tc.tile_pool(name="wts", bufs=1) as wpool,
            tc.tile_pool(name="hstage", bufs=4) as hstage,
            tc.tile_pool(name="htile", bufs=2) as htile,
            tc.tile_pool(name="qkv", bufs=2) as qkv,
            tc.tile_pool(name="vstage", bufs=2) as vsp,
            tc.tile_pool(name="probs", bufs=4) as probsp,
            tc.tile_pool(name="csbp", bufs=2) as csbp,
            tc.tile_pool(name="recipp", bufs=2) as recipp,
            tc.tile_pool(name="recipbp", bufs=2) as recipbp,
            tc.tile_pool(name="ctxn", bufs=2) as ctxp,
            tc.tile_pool(name="ostage", bufs=4) as ostage,
            tc.tile_pool(name="ps_scores", bufs=2, space="PSUM") as ps_s,
            tc.tile_pool(name="ps_ctx", bufs=1, space="PSUM") as ps_c,
            tc.tile_pool(name="ps_misc", bufs=2, space="PSUM") as ps_m,
        ):
            # ---- constants ----
            ident32 = cpool.tile([P, P], F32)
            make_identity(nc, ident32[:])
            identr = cpool.tile([P, P], F32R)
            nc.vector.tensor_copy(identr[:], ident32[:])
            ones32 = cpool.tile([P, 1], F32)
            nc.gpsimd.memset(ones32[:], 1.0)
            bq_t = cpool.tile([P, 1], F32, tag="bq")
            bk_t = cpool.tile([P, 1], F32, tag="bk")
            bv_t = cpool.tile([P, 1], F32, tag="bv")
            nc.sync.dma_start(bq_t[:], bq.rearrange("(p o) -> p o", o=1))
            nc.sync.dma_start(bk_t[:], bk.rearrange("(p o) -> p o", o=1))
            nc.sync.dma_start(bv_t[:], bv.rearrange("(p o) -> p o", o=1))

            # ---- weight transposes: W[f,e] -> W^T chunks [e(128), f] ----
            def transpose_weight(w_ap, name):
                wt = wpool.tile([P, EC, F], F32R, tag=f"{name}T")
                st = wpool.tile([P, D], F32, tag="wstage")
                nc.sync.dma_start(st[:], w_ap)
                for eg in range(2):          # 2 groups of 4 chunks
                    pt = ps_m.tile([P, 512], F32, tag="misc")
                    for i in range(4):
                        e = eg * 4 + i
                        nc.tensor.transpose(pt[:, bass.ts(i, P)],
                                            st[:, bass.ts(e, P)], ident32[:])
                    nc.vector.tensor_copy(
                        wt[:, bass.ts(eg, 4)],
                        pt[:].rearrange("p (i f) -> p i f", i=4))
                return wt

            wqT = transpose_weight(wq, "wq")
            wkT = transpose_weight(wk, "wk")
            wvT = transpose_weight(wv, "wv")

            # wo [D, F] -> woT [d(128), fo 1024]
            woT = wpool.tile([P, D], F32R, tag="woT")
            for eg in range(2):
                pt = ps_m.tile([P, 512], F32, tag="misc")
                for i in range(4):
                    e = eg * 4 + i
                    st = wpool.tile([P, F], F32, tag="wostage")
                    nc.sync.dma_start(st[:], wo[bass.ts(e, P), :])
                    nc.tensor.transpose(pt[:, bass.ts(i, P)], st[:],
                                        ident32[:])
                nc.vector.tensor_copy(woT[:, bass.ts(eg, 512)], pt[:])

            # ---- per-batch pipeline ----
            for b in range(B):
                qT = qkv.tile([P, S], F32R, tag="qT")
                kT = qkv.tile([P, S], F32R, tag="kT")
                # vaug[tok, chunk, head, 0:64] = V,
                # vaug[tok, chunk, head, 64] = 1 (denominator col)
                vaug = qkv.tile([P, KC, 2, 65], F32R, tag="vaug")
                nc.vector.tensor_copy(
                    vaug[:, :, :, 64:65],
                    ones32[:, None, None, :].to_broadcast((P, KC, 2, 1)),
                )

                for tt in range(TT):
                    # h^T for this 512-token tile; batch 4 transposes per CAST
                    ht = htile.tile([P, EC, 512], F32R)
                    hss = []
                    for p in range(4):
                        hs = hstage.tile([P, D], F32)
                        nc.sync.dma_start(
                            hs[:], hid[b, tt * 512 + p * P:
                                       tt * 512 + (p + 1) * P, :])
                        hss.append(hs)
                    for e in range(EC):
                        pt = ps_m.tile([P, 512], F32, tag="misc")
                        for p in range(4):
                            nc.tensor.transpose(pt[:, bass.ts(p, P)],
                                                hss[p][:, bass.ts(e, P)],
                                                ident32[:])
                        nc.vector.tensor_copy(ht[:, e], pt[:])

                    # projections for this token tile
                    def project(wT):
                        pp = ps_m.tile([P, 512], F32, tag="misc")
                        for e in range(EC):
                            nc.tensor.matmul(pp[:], wT[:, e], ht[:, e],
                                             start=(e == 0), stop=(e == EC - 1))
                        return pp

                    pp = project(wqT)
                    nc.vector.tensor_scalar(
                        qT[:, bass.ts(tt, 512)], pp[:], bq_t[:], 0.125,
                        mybir.AluOpType.add, mybir.AluOpType.mult)
                    pp = project(wkT)
                    nc.vector.tensor_scalar_add(
                        kT[:, bass.ts(tt, 512)], pp[:], bk_t[:])
                    pp = project(wvT)
                    vt = vsp.tile([P, 512], F32R)
                    nc.vector.tensor_scalar_add(vt[:], pp[:], bv_t[:])
                    # V^T -> V; 4 transposes (token blocks) into one psum,
                    # then a single strided CAST into vaug
                    pt = ps_m.tile([P, 512], F32R, tag="misc")
                    for p in range(4):
                        nc.tensor.transpose(pt[:, bass.ts(p, P)],
                                            vt[:, bass.ts(p, P)], identr[:])
                    nc.vector.tensor_copy(
                        vaug[:, bass.ts(tt, 4), :, 0:64],
                        pt[:].rearrange("p (c h f) -> p c h f", c=4, h=2))

                ctxn = ctxp.tile([P, S], F32R)

                # ---- attention: both heads row-packed per q-block ----
                # Head A uses PE row-groups 0-1 (partitions 0:64), head B
                # rows 2-3 (64:128); their score matmuls run concurrently
                # in the array. q-block = 512 keeps PSUM in budget:
                # scores 2x2 banks + ctx 2 banks + misc 2 = 8.
                for qb in range(4):
                    q0 = qb * 512
                    pscA = ps_c.tile([65, 512], F32, tag="pscA")
                    pscB = ps_c.tile([65, 512], F32, tag="pscB")
                    prev = None
                    for c in range(KC):
                        pssA = ps_s.tile([P, 512], F32, tag="pssA")
                        pssB = ps_s.tile([P, 512], F32, tag="pssB")
                        nc.tensor.matmul(pssA[:], kT[0:64, bass.ts(c, P)],
                                         qT[0:64, q0:q0 + 512],
                                         start=True, stop=True)
                        nc.tensor.matmul(pssB[:], kT[64:128, bass.ts(c, P)],
                                         qT[64:128, q0:q0 + 512],
                                         start=True, stop=True)
                        prA = probsp.tile([P, 512], F32R, tag="prA")
                        nc.scalar.activation(prA[:], pssA[:], EXP)
                        prB = probsp.tile([P, 512], F32R, tag="prB")
                        nc.scalar.activation(prB[:], pssB[:], EXP)
                        if prev is not None:
                            pA, pB, cc = prev
                            nc.tensor.matmul(pscA[:], vaug[:, cc, 0, :],
                                             pA[:], start=(cc == 0),
                                             stop=False)
                            nc.tensor.matmul(pscB[:], vaug[:, cc, 1, :],
                                             pB[:], start=(cc == 0),
                                             stop=False)
                        prev = (prA, prB, c)
                    pA, pB, cc = prev
                    nc.tensor.matmul(pscA[:], vaug[:, cc, 0, :], pA[:],
                                     start=False, stop=True)
                    nc.tensor.matmul(pscB[:], vaug[:, cc, 1, :], pB[:],
                                     start=False, stop=True)

                    for h, psc in ((0, pscA), (1, pscB)):
                        hs_lo, hs_hi = h * 64, (h + 1) * 64
                        csb = csbp.tile([65, 512], F32)
                        nc.vector.tensor_copy(csb[:], psc[:])
                        rc = recipp.tile([1, 512], F32)
                        nc.gpsimd.dma_start(rc[0:1, :], csb[64:65, :])
                        rb = recipbp.tile([64, 512], F32)
                        nc.gpsimd.partition_broadcast(rb[:], rc[0:1, :])
                        nc.vector.reciprocal_approx_fast(rb[:], rb[:])
                        nc.vector.tensor_tensor(
                            ctxn[hs_lo:hs_hi, q0:q0 + 512],
                            csb[0:64, :], rb[:], mybir.AluOpType.mult)

                # ---- partial output projection for batch b ----
                for st in range(S // P):
                    for fo in range(2):
                        po = ps_m.tile([P, 512], F32, tag="misc")
                        nc.tensor.matmul(po[:], ctxn[:, bass.ts(st, P)],
                                         woT[:, bass.ts(fo, 512)],
                                         start=True, stop=True)
                        ot = ostage.tile([P, 512], F32)
                        nc.vector.tensor_copy(ot[:], po[:])
                        nc.scalar.dma_start(
                            out[b, bass.ts(st, P), bass.ts(fo, 512)], ot[:])

    nc.compile()
    return nc


_NC_CACHE = None


def build_in_maps(hid, Wq, bq, Wk, bk, Wv, bv, Wo):
    hid = np.ascontiguousarray(np.asarray(hid, np.float32))
    in_maps = []
    for c in range(NCORES):
        sl = slice(c * F, (c + 1) * F)
        in_maps.append({
            "hid": hid,
            "wq": np.ascontiguousarray(np.asarray(Wq, np.float32)[sl]),
            "wk": np.ascontiguousarray(np.asarray(Wk, np.float32)[sl]),
            "wv": np.ascontiguousarray(np.asarray(Wv, np.float32)[sl]),
            "wo": np.ascontiguousarray(np.asarray(Wo, np.float32)[:, sl]),
            "bq": np.ascontiguousarray(np.asarray(bq, np.float32)[sl]),
            "bk": np.ascontiguousarray(np.asarray(bk, np.float32)[sl]),
            "bv": np.ascontiguousarray(np.asarray(bv, np.float32)[sl]),
        })
    return in_maps


def kernel(hidden_states, Wq, bq, Wk, bk, Wv, bv, Wo, bo):
    global _NC_CACHE
    if _NC_CACHE is None:
        _NC_CACHE = build_nc()
    nc = _NC_CACHE

    hid = np.ascontiguousarray(np.asarray(hidden_states, dtype=np.float32))
    Wq = np.asarray(Wq, dtype=np.float32)
    Wk = np.asarray(Wk, dtype=np.float32)
    Wv = np.asarray(Wv, dtype=np.float32)
    Wo = np.asarray(Wo, dtype=np.float32)

    in_maps = build_in_maps(hid, Wq, bq, Wk, bk, Wv, bv, Wo)

    try:
        res = bass_utils.run_bass_kernel_spmd(nc, in_maps,
                                              core_ids=list(range(NCORES)))
    except Exception:
        # transient device flake (e.g. NRT_EXEC_UNIT_UNRECOVERABLE): retry once
        res = bass_utils.run_bass_kernel_spmd(nc, in_maps,
                                              core_ids=list(range(NCORES)))
    acc = res.results[0]["out"].astype(np.float32).copy()
    for c in range(1, NCORES):
        acc += res.results[c]["out"]
    acc += np.asarray(bo, dtype=np.float32)
    return acc


# revision 25
# speedup vs baseline: 1.1605x; 1.1605x over previous
"""Multi-head attention Trainium2 Bass kernel.

Problem: B=4, S=2048, H=16, DH=64, D=1024, fp32.
  q/k/v = hidden @ W{q,k,v}.T + b; scores = q k^T / 8; probs = softmax;
  ctx = probs v; out = ctx @ Wo.T + bo.

Sharding: tensor-parallel over heads. Core c owns heads (2c, 2c+1) for all
4 batches: column-slices of Wq/Wk/Wv (torch [out,in] layout -> row slices),
row-slice of Wo. Each core computes a partial output projection over its
128 context features; host sums the 8 partials and adds bo.

Per-core data layout (all matmuls in float32r, TF32-grade, full PE rate):
  h^T   [e=1024 (8 chunks of 128p), tok]  via PE transpose of hidden tiles
  Q^T/K^T [f=128 (2 heads x 64), tok]     = Wslice^T-lhsT x h^T-rhs
  Vaug  [tok (16 chunks of 128p), chunk, head, 65] (ones col + 64 V cols)
  S^T   [kt chunk 128p, q 1024]  = K^T-lhsT x Q^T-rhs     (PSUM)
  P     = exp(S^T)  (no max subtraction needed: scores ~ N(0,1))  (SBUF f32r)
  den/ctx^T = Vaug-lhsT x P-rhs accumulated over kt chunks (PSUM [65, q])
  ctxn^T = ctx^T * (1/den)  (den broadcast over partitions via GpSimd)
  out_partial [tok, fo] = ctxn^T-lhsT x Wo^T-rhs
"""
import numpy as np

import concourse.bass as bass
import concourse.tile as tile
from concourse import bacc, mybir
from concourse import bass_utils
from concourse.masks import make_identity

F32 = mybir.dt.float32
F32R = mybir.dt.float32r
EXP = mybir.ActivationFunctionType.Exp

B = 4
S = 2048
D = 1024
NCORES = 8
P = 128          # partitions
EC = D // P      # 8 e-chunks
TT = S // 512    # 4 token tiles per batch
KC = S // P      # 16 kt chunks
QB = S // 1024   # 2 q blocks per head
F = 128          # per-core feature slice (2 heads x 64)


def build_nc():
    nc = bacc.Bacc("TRN2", target_bir_lowering=False, debug=False,
                   enable_asserts=True, num_devices=NCORES)

    hid = nc.dram_tensor("hid", [B, S, D], F32, kind="ExternalInput").ap()
    wq = nc.dram_tensor("wq", [F, D], F32, kind="ExternalInput").ap()
    wk = nc.dram_tensor("wk", [F, D], F32, kind="ExternalInput").ap()
    wv = nc.dram_tensor("wv", [F, D], F32, kind="ExternalInput").ap()
    wo = nc.dram_tensor("wo", [D, F], F32, kind="ExternalInput").ap()
    bq = nc.dram_tensor("bq", [F], F32, kind="ExternalInput").ap()
    bk = nc.dram_tensor("bk", [F], F32, kind="ExternalInput").ap()
    bv = nc.dram_tensor("bv", [F], F32, kind="ExternalInput").ap()
    out = nc.dram_tensor("out", [B, S, D], F32, kind="ExternalOutput").ap()

    with tile.TileContext(nc) as tc:
        with (
            tc.tile_pool(name="const", bufs=1) as cpool,
            tc.tile_pool(name="wts", bufs=1) as wpool,
            tc.tile_pool(name="hstage", bufs=4) as hstage,
            tc.tile_pool(name="htile", bufs=2) as htile,
            tc.tile_pool(name="qkv", bufs=2) as qkv,
            tc.tile_pool(name="vstage", bufs=2) as vsp,
            tc.tile_pool(name="probs", bufs=4) as probsp,
            tc.tile_pool(name="csbp", bufs=2) as csbp,
            tc.tile_pool(name="recipp", bufs=2) as recipp,
            tc.tile_pool(name="recipbp", bufs=2) as recipbp,
            tc.tile_pool(name="ctxn", bufs=2) as ctxp,
            tc.tile_pool(name="ostage", bufs=4) as ostage,
            tc.tile_pool(name="ps_scores", bufs=2, space="PSUM") as ps_s,
            tc.tile_pool(name="ps_ctx", bufs=1, space="PSUM") as ps_c,
            tc.tile_pool(name="ps_misc", bufs=2, space="PSUM") as ps_m,
        ):
            # ---- constants ----
            ident32 = cpool.tile([P, P], F32)
            make_identity(nc, ident32[:])
            identr = cpool.tile([P, P], F32R)
            nc.vector.tensor_copy(identr[:], ident32[:])
            ones32 = cpool.tile([P, 1], F32)
            nc.gpsimd.memset(ones32[:], 1.0)
            bq_t = cpool.tile([P, 1], F32, tag="bq")
            bk_t = cpool.tile([P, 1], F32, tag="bk")
            bv_t = cpool.tile([P, 1], F32, tag="bv")
            nc.sync.dma_start(bq_t[:], bq.rearrange("(p o) -> p o", o=1))
            nc.sync.dma_start(bk_t[:], bk.rearrange("(p o) -> p o", o=1))
            nc.sync.dma_start(bv_t[:], bv.rearrange("(p o) -> p o", o=1))

            # ---- weight transposes: W[f,e] -> W^T chunks [e(128), f] ----
            def transpose_weight(w_ap, name):
                wt = wpool.tile([P, EC, F], F32R, tag=f"{name}T")
                st = wpool.tile([P, D], F32, tag="wstage")
                nc.sync.dma_start(st[:], w_ap)
                for eg in range(2):          # 2 groups of 4 chunks
                    pt = ps_m.tile([P, 512], F32, tag="misc")
                    for i in range(4):
                        e = eg * 4 + i
                        nc.tensor.transpose(pt[:, bass.ts(i, P)],
                                            st[:, bass.ts(e, P)], ident32[:])
                    nc.vector.tensor_copy(
                        wt[:, bass.ts(eg, 4)],
                        pt[:].rearrange("p (i f) -> p i f", i=4))
                return wt

            wqT = transpose_weight(wq, "wq")
            wkT = transpose_weight(wk, "wk")
            wvT = transpose_weight(wv, "wv")

            # wo [D, F] -> woT [d(128), fo 1024]
            woT = wpool.tile([P, D], F32R, tag="woT")
            for eg in range(2):
                pt = ps_m.tile([P, 512], F32, tag="misc")
                for i in range(4):
                    e = eg * 4 + i
                    st = wpool.tile([P, F], F32, tag="wostage")
                    nc.sync.dma_start(st[:], wo[bass.ts(e, P), :])
                    nc.tensor.transpose(pt[:, bass.ts(i, P)], st[:],
                                        ident32[:])
                nc.vector.tensor_copy(woT[:, bass.ts(eg, 512)], pt[:])

            # ---- per-batch pipeline ----
            for b in range(B):
                qT = qkv.tile([P, S], F32R, tag="qT")
                kT = qkv.tile([P, S], F32R, tag="kT")
                # vaug[tok, chunk, head, 0:64] = V,
                # vaug[tok, chunk, head, 64] = 1 (denominator col)
                vaug = qkv.tile([P, KC, 2, 65], F32R, tag="vaug")
                nc.vector.tensor_copy(
                    vaug[:, :, :, 64:65],
                    ones32[:, None, None, :].to_broadcast((P, KC, 2, 1)),
                )

                for tt in range(TT):
                    # h^T for this 512-token tile; batch 4 transposes per CAST
                    ht = htile.tile([P, EC, 512], F32R)
                    hss = []
                    for p in range(4):
                        hs = hstage.tile([P, D], F32)
                        nc.sync.dma_start(
                            hs[:], hid[b, tt * 512 + p * P:
                                       tt * 512 + (p + 1) * P, :])
                        hss.append(hs)
                    for e in range(EC):
                        pt = ps_m.tile([P, 512], F32, tag="misc")
                        for p in range(4):
                            nc.tensor.transpose(pt[:, bass.ts(p, P)],
                                                hss[p][:, bass.ts(e, P)],
                                                ident32[:])
                        nc.vector.tensor_copy(ht[:, e], pt[:])

                    # projections for this token tile
                    def project(wT):
                        pp = ps_m.tile([P, 512], F32, tag="misc")
                        for e in range(EC):
                            nc.tensor.matmul(pp[:], wT[:, e], ht[:, e],
                                             start=(e == 0), stop=(e == EC - 1))
                        return pp

                    pp = project(wqT)
                    nc.vector.tensor_scalar(
                        qT[:, bass.ts(tt, 512)], pp[:], bq_t[:], 0.125,
                        mybir.AluOpType.add, mybir.AluOpType.mult)
                    pp = project(wkT)
                    nc.vector.tensor_scalar_add(
                        kT[:, bass.ts(tt, 512)], pp[:], bk_t[:])
                    pp = project(wvT)
                    vt = vsp.tile([P, 512], F32R)
                    nc.vector.tensor_scalar_add(vt[:], pp[:], bv_t[:])
                    # V^T -> V; 4 transposes (token blocks) into one psum,
                    # then a single strided CAST into vaug
                    pt = ps_m.tile([P, 512], F32R, tag="misc")
                    for p in range(4):
                        nc.tensor.transpose(pt[:, bass.ts(p, P)],
                                            vt[:, bass.ts(p, P)], identr[:])
                    nc.vector.tensor_copy(
                        vaug[:, bass.ts(tt, 4), :, 0:64],
                        pt[:].rearrange("p (c h f) -> p c h f", c=4, h=2))

                ctxn = ctxp.tile([P, S], F32R)

                # ---- attention, head h, q-block qb ----
                for h in range(2):
                    hs_lo, hs_hi = h * 64, (h + 1) * 64
                    for qb in range(QB):
                        q0 = qb * 1024
                        psc = ps_c.tile([65, 1024], F32)
                        prev = None
                        for c in range(KC):
                            pss = ps_s.tile([P, 1024], F32)
                            for qt in range(2):
                                nc.tensor.matmul(
                                    pss[:, bass.ts(qt, 512)],
                                    kT[hs_lo:hs_hi, bass.ts(c, P)],
                                    qT[hs_lo:hs_hi,
                                       q0 + qt * 512: q0 + (qt + 1) * 512],
                                    start=True, stop=True)
                            pr = probsp.tile([P, 1024], F32R)
                            nc.scalar.activation(pr[:], pss[:], EXP)
                            # ctx matmuls for the PREVIOUS chunk (sw pipeline:
                            # keeps PE from stalling on the current exp)
                            if prev is not None:
                                pv, cc = prev
                                for qt in range(2):
                                    nc.tensor.matmul(
                                        psc[:, bass.ts(qt, 512)],
                                        vaug[:, cc, h, :],
                                        pv[:, bass.ts(qt, 512)],
                                        start=(cc == 0), stop=False)
                            prev = (pr, c)
                        pv, cc = prev
                        for qt in range(2):
                            nc.tensor.matmul(
                                psc[:, bass.ts(qt, 512)],
                                vaug[:, cc, h, :],
                                pv[:, bass.ts(qt, 512)],
                                start=False, stop=True)

                        # copy den+ctx to SBUF first: releases ps_ctx after
                        # ~1.2us so the next unit's PE work isn't gated on
                        # the reciprocal chain (keeps HAM warm)
                        csb = csbp.tile([65, 1024], F32)
                        nc.vector.tensor_copy(csb[:], psc[:])
                        # hop the denom row from partition 64 to partition 0
                        # via sbuf->sbuf DMA (engines can't shift partitions),
                        # then broadcast + reciprocal lane-aligned
                        rc = recipp.tile([1, 1024], F32)
                        nc.gpsimd.dma_start(rc[0:1, :], csb[64:65, :])
                        rb = recipbp.tile([64, 1024], F32)
                        nc.gpsimd.partition_broadcast(rb[:], rc[0:1, :])
                        nc.vector.reciprocal_approx_fast(rb[:], rb[:])
                        nc.vector.tensor_tensor(
                            ctxn[hs_lo:hs_hi, q0:q0 + 1024],
                            csb[0:64, :], rb[:], mybir.AluOpType.mult)

                # ---- partial output projection for batch b ----
                for st in range(S // P):
                    for fo in range(2):
                        po = ps_m.tile([P, 512], F32, tag="misc")
                        nc.tensor.matmul(po[:], ctxn[:, bass.ts(st, P)],
                                         woT[:, bass.ts(fo, 512)],
                                         start=True, stop=True)
                        ot = ostage.tile([P, 512], F32)
                        nc.vector.tensor_copy(ot[:], po[:])
                        nc.scalar.dma_start(
                            out[b, bass.ts(st, P), bass.ts(fo, 512)], ot[:])

    nc.compile()
    return nc


_NC_CACHE = None


def build_in_maps(hid, Wq, bq, Wk, bk, Wv, bv, Wo):
    hid = np.ascontiguousarray(np.asarray(hid, np.float32))
    in_maps = []
    for c in range(NCORES):
        sl = slice(c * F, (c + 1) * F)
        in_maps.append({
            "hid": hid,
            "wq": np.ascontiguousarray(np.asarray(Wq, np.float32)[sl]),
            "wk": np.ascontiguousarray(np.asarray(Wk, np.float32)[sl]),
            "wv": np.ascontiguousarray(np.asarray(Wv, np.float32)[sl]),
            "wo": np.ascontiguousarray(np.asarray(Wo, np.float32)[:, sl]),
            "bq": np.ascontiguousarray(np.asarray(bq, np.float32)[sl]),
            "bk": np.ascontiguousarray(np.asarray(bk, np.float32)[sl]),
            "bv": np.ascontiguousarray(np.asarray(bv, np.float32)[sl]),
        })
    return in_maps


def kernel(hidden_states, Wq, bq, Wk, bk, Wv, bv, Wo, bo):
    global _NC_CACHE
    if _NC_CACHE is None:
        _NC_CACHE = build_nc()
    nc = _NC_CACHE

    hid = np.ascontiguousarray(np.asarray(hidden_states, dtype=np.float32))
    Wq = np.asarray(Wq, dtype=np.float32)
    Wk = np.asarray(Wk, dtype=np.float32)
    Wv = np.asarray(Wv, dtype=np.float32)
    Wo = np.asarray(Wo, dtype=np.float32)

    in_maps = build_in_maps(hid, Wq, bq, Wk, bk, Wv, bv, Wo)

    try:
        res = bass_utils.run_bass_kernel_spmd(nc, in_maps,
                                              core_ids=list(range(NCORES)))
    except Exception:
        # transient device flake (e.g. NRT_EXEC_UNIT_UNRECOVERABLE): retry once
        res = bass_utils.run_bass_kernel_spmd(nc, in_maps,
                                              core_ids=list(range(NCORES)))
    acc = res.results[0]["out"].astype(np.float32).copy()
    for c in range(1, NCORES):
        acc += res.results[c]["out"]
    acc += np.asarray(bo, dtype=np.float32)
    return acc
